# revision 14
# baseline (speedup 1.0000x reference)
"""Distributed MQA causal attention for TRN2 (8 NeuronCores).

Sharding: 8 cores = 2 (batch) x 4 (head-group tensor parallel).
Core c handles batch b=c//4, head group g=c%4 (8 heads, o-slice of 1024).

K/V projection is t-sharded across the 4 cores of a batch group: each core
receives an extra input `xkv` (its 512-row slice of x), computes K/V for
that chunk once, and one AllGather assembles the full K^T / V for all
chunks (vs. every core computing all of K/V).

After attention, the per-core attn^T chunks are AllGather-ed (groups of 4)
and each core computes a 1024-wide column slice of the output projection.
The chunk-3 Q projection is hoisted into the chunk-2 body so wq^T dies
early; wo^T (aliased on wq^T) then loads during chunk-2 attention and the
output projections overlap chunk-3 attention instead of serializing after.

All matmuls run in bf16 (f32 accumulation in PSUM).  DMA ring plan (each
DGE ring is FIFO, so each gets a stream whose deadlines are monotone):
  gpsimd (SWDGE, casts): xkv, wk, wv, wq (streamed), wo
  scalar (HWDGE):        all x f32 loads + DVE bf16 casts; x_bf stores
  sync   (HWDGE):        xT DMA-transposes, kv unpack, wo_bf stores, woT,
                         attn ships, gT loads, out writes
"""

import numpy as np

import concourse.bass as bass
import concourse.mybir as mybir
import concourse.tile as tile
from concourse import bacc
from concourse.bass_utils import run_bass_kernel_spmd
from concourse.masks import make_identity

# Problem shape (hardcoded; kernel.py must be self-contained).
B, T, D = 2, 2048, 4096
H, HD = 32, 128
NCORES, TPG = 8, 4
HL = H // TPG            # 8 local heads per core
OL = HL * HD             # 1024 local q/o dims per core
P = 128
TC = 512                 # t-chunk width (moving-dim of the big GEMMs)
NTC = T // TC            # 4
ND = D // P              # 32 contraction tiles for D
NT = T // P              # 16 k-tiles
QW = 512                 # staging width for f32 x loads
NQ = D // QW             # 8 pieces per row-tile
SCALE = float(1.0 / np.sqrt(HD))

BF16 = mybir.dt.bfloat16
F32 = mybir.dt.float32

_CACHE = {}
LAST_RESULT = None  # BassKernelResults of the most recent run (for test harness)


def build_nc():
    nc = bacc.Bacc(None, target_bir_lowering=False, num_devices=NCORES)

    x_ext = nc.declare_dram_parameter("x", [T, D], F32, isOutput=False)
    xkv_ext = nc.declare_dram_parameter("xkv", [TC, D], F32, isOutput=False)
    wq_ext = nc.declare_dram_parameter("wq", [OL, D], F32, isOutput=False)
    bq_ext = nc.declare_dram_parameter("bq", [OL], F32, isOutput=False)
    wk_ext = nc.declare_dram_parameter("wk", [HD, D], F32, isOutput=False)
    bk_ext = nc.declare_dram_parameter("bk", [HD], F32, isOutput=False)
    wv_ext = nc.declare_dram_parameter("wv", [HD, D], F32, isOutput=False)
    bv_ext = nc.declare_dram_parameter("bv", [HD], F32, isOutput=False)
    wo_ext = nc.declare_dram_parameter("wo", [OL, D], F32, isOutput=False)
    bo_ext = nc.declare_dram_parameter("bo", [OL], F32, isOutput=False)
    out_ext = nc.declare_dram_parameter("out", [T, OL], F32, isOutput=True)

    with tile.TileContext(nc) as tc:
        with (
            tc.tile_pool(name="consts", bufs=1) as consts,
            tc.tile_pool(name="wpool", bufs=1) as wpool,
            tc.tile_pool(name="wsmall", bufs=1) as wsmall,
            tc.tile_pool(name="slab", bufs=2) as slabp,
            tc.tile_pool(name="nat", bufs=2) as natp,
            tc.tile_pool(name="f32c", bufs=2) as f32cp,
            tc.tile_pool(name="bcc", bufs=2) as bccp,
            tc.tile_pool(name="big", bufs=1) as bigp,
            tc.tile_pool(name="qtc", bufs=2) as qtcp,
            tc.tile_pool(name="atc", bufs=1) as atcp,
            tc.tile_pool(name="esb", bufs=2) as esbp,
            tc.tile_pool(name="tmp", bufs=3) as tmpp,
            tc.tile_pool(name="psum", bufs=1, space="PSUM") as psump,
            tc.tile_pool(name="dram", bufs=1, space="DRAM") as dram,
        ):
            # ---- Constants (tiny, first so gpsimd builds them before casts)
            ident = consts.tile([P, P], BF16)
            make_identity(nc, ident[:])
            # Diagonal causal 0/1 mask: mask0[x, y] = 1 if y >= x else 0.
            mask0 = consts.tile([P, TC], BF16, name="mask0")
            nc.gpsimd.memset(mask0[:], 1.0)
            nc.gpsimd.affine_select(
                out=mask0[:],
                in_=mask0[:],
                pattern=[[1, TC]],
                compare_op=mybir.AluOpType.is_ge,
                fill=0.0,
                base=0,
                channel_multiplier=-1,
            )
            bq_sb = consts.tile([P, HL], F32)
            nc.sync.dma_start(bq_sb[:], bq_ext[:].rearrange("(o p) -> p o", p=P))
            bk_sb = consts.tile([P, 1], F32)
            nc.sync.dma_start(bk_sb[:], bk_ext[:].rearrange("(o p) -> p o", p=P))
            bv_sb = consts.tile([P, 1], F32)
            nc.sync.dma_start(bv_sb[:], bv_ext[:].rearrange("(o p) -> p o", p=P))
            bo_row = natp.tile([1, OL], BF16, tag="nat", name="bo_row")
            nc.gpsimd.dma_start(bo_row[:], bo_ext[None, :])
            bo_bc = consts.tile([P, OL], BF16)
            nc.gpsimd.partition_broadcast(bo_bc[:], bo_row[:])

            # Persistent attention operands.
            kT = bigp.tile([P, T], BF16, name="kT")               # [hd, t]
            vaug = bigp.tile([P, NT, HD + 1], BF16, name="vaug")  # [tk, kt, 129]
            nc.vector.memset(vaug[:, :, HD : HD + 1], 1.0)

            wqT = wpool.tile([P, ND, OL], BF16, tag="bigw", name="wqT")
            wkT = wsmall.tile([P, ND, HD], BF16, name="wkT")
            wvT = wsmall.tile([P, ND, HD], BF16, name="wvT")

            # ---- gpsimd cast loads (SWDGE) + PE transpose --------------------
            def load_T_gpsimd(src_ext, rows, dstT, col0, what):
                for blk in range(rows // P):
                    nat = natp.tile([P, D], BF16, tag="nat", name=f"nat_{what}{blk}")
                    nc.gpsimd.dma_start(nat[:], src_ext[blk * P : (blk + 1) * P, :])
                    for g in range(0, ND, 4):
                        pst = psump.tile(
                            [P, 4 * P], BF16, tag="attn", bufs=4, name=f"ptr_{what}{blk}{g}"
                        )
                        for j in range(4):
                            nc.tensor.transpose(
                                pst[:, j * P : (j + 1) * P],
                                nat[:, (g + j) * P : (g + j + 1) * P],
                                ident[:],
                            )
                        nc.vector.tensor_copy(
                            dstT[:, g : g + 4, col0 + blk * P : col0 + (blk + 1) * P],
                            pst[:].rearrange("p (g t) -> p g t", g=4),
                        )

            # ---- scalar f32 loads + DVE cast (+ optional scalar-ring store) --
            def load_cast_x(r0, what, store_dst=None):
                pieces = []
                for qq in range(NQ):
                    f32t = f32cp.tile([P, QW], F32, tag="f32c", name=f"f_{what}{qq}")
                    nc.scalar.dma_start(
                        f32t[:], x_ext[r0 : r0 + P, qq * QW : (qq + 1) * QW]
                    )
                    bft = bccp.tile([P, QW], BF16, tag="bcc", name=f"b_{what}{qq}")
                    nc.vector.tensor_copy(bft[:], f32t[:])
                    if store_dst is not None:
                        nc.scalar.dma_start(
                            store_dst[r0 : r0 + P, qq * QW : (qq + 1) * QW], bft[:]
                        )
                    pieces.append(bft)
                return pieces

            def transpose_pieces(pieces, dstT, col0, what):
                for qq, bft in enumerate(pieces):
                    base_dt = qq * (QW // P)
                    pst = psump.tile(
                        [P, 4 * P], BF16, tag="attn", bufs=4, name=f"ptx_{what}{qq}"
                    )
                    for j in range(QW // P):
                        nc.tensor.transpose(
                            pst[:, j * P : (j + 1) * P],
                            bft[:, j * P : (j + 1) * P],
                            ident[:],
                        )
                    nc.vector.tensor_copy(
                        dstT[:, base_dt : base_dt + QW // P, col0 : col0 + P],
                        pst[:, 0 : QW].rearrange("p (g t) -> p g t", g=QW // P),
                    )

            # ---- DRAM bf16 scratch for late operands (x1-3, wo) --------------
            x_bf = dram.tile([T, D], BF16)
            wo_bf = dram.tile([OL, D], BF16)

            # KV AllGather buffers: in [128, 1024] = [kT piece | v nat piece]
            cc_kv_in = dram.tile([P, 2 * TC], BF16, name="cc_kv_in")
            cc_kv_g = dram.tile([TPG * P, 2 * TC], BF16, name="cc_kv_g")

            # attn AllGather buffers, one per t-chunk (column-sliced attn^T).
            cc_in = [dram.tile([OL, TC], BF16, name=f"cc_in{c}") for c in range(NTC)]
            cc_g = [
                dram.tile([TPG * OL, TC], BF16, name=f"cc_g{c}") for c in range(NTC)
            ]
            HH = OL // 2  # 512 rows = 4 heads
            cc_in3 = [dram.tile([HH, TC], BF16, name=f"cc_in3{i}") for i in range(2)]
            cc_g3 = [dram.tile([TPG * HH, TC], BF16, name=f"cc_g3{i}") for i in range(2)]

            woT = wpool.tile([P, ND, OL], BF16, tag="bigw", name="woT")

            # ========== Stage A: sharded K/V projection + gather ==============
            xkvT = slabp.tile([P, ND, TC], BF16, tag="slab", name="xkvT")
            load_T_gpsimd(xkv_ext, TC, xkvT, 0, "xkv")
            load_T_gpsimd(wk_ext, HD, wkT, 0, "wk")
            load_T_gpsimd(wv_ext, HD, wvT, 0, "wv")

            # Pre-issue the first two wq head loads so they queue ahead of the
            # kv ship on the gpsimd ring (the ship waits on the kv matmuls).
            nat_q = {}
            for oo in range(2):
                nat_q[oo] = natp.tile([P, D], BF16, tag="nat", name=f"natq{oo}")
                nc.gpsimd.dma_start(nat_q[oo][:], wq_ext[oo * P : (oo + 1) * P, :])

            for which, wT, b_sb in (("k", wkT, bk_sb), ("v", wvT, bv_sb)):
                ps = psump.tile([P, TC], F32, tag="mm512", bufs=2, name=f"pskv_{which}")
                for dt in range(ND):
                    nc.tensor.matmul(
                        ps[:],
                        wT[:, dt, :],
                        xkvT[:, dt, :],
                        start=(dt == 0),
                        stop=(dt == ND - 1),
                    )
                if which == "k":
                    kpc = tmpp.tile([P, TC], BF16, tag="vt", bufs=1, name="kpc")
                    nc.vector.tensor_scalar_add(kpc[:], ps[:], b_sb[:])
                    nc.gpsimd.dma_start(cc_kv_in[:, 0:TC], kpc[:])
                else:
                    vt = tmpp.tile([P, TC], BF16, tag="vt", bufs=1, name="vt_kv")
                    nc.vector.tensor_scalar_add(vt[:], ps[:], b_sb[:])
                    pstv = psump.tile([P, 4 * P], BF16, tag="attn", bufs=4, name="pstv")
                    for jj in range(TC // P):
                        nc.tensor.transpose(
                            pstv[:, jj * P : (jj + 1) * P],
                            vt[:, jj * P : (jj + 1) * P],
                            ident[:],
                        )
                    vnat = tmpp.tile([P, TC], BF16, tag="vt", bufs=1, name="vnat")
                    nc.vector.tensor_copy(vnat[:], pstv[:])
                    nc.gpsimd.dma_start(cc_kv_in[:, TC : 2 * TC], vnat[:])

            nc.gpsimd.collective_compute(
                "AllGather",
                mybir.AluOpType.bypass,
                replica_groups=[[0, 1, 2, 3], [4, 5, 6, 7]],
                ins=[cc_kv_in[:, :].opt()],
                outs=[cc_kv_g[:, :].opt()],
            )

            def emit_unpack_kv():
                # kT[hd, t] and vaug[tk, kt, hd] from the gathered buffer.
                nc.sync.dma_start(
                    kT[:, :].rearrange("p (c t) -> p c t", c=NTC),
                    cc_kv_g[:, 0:TC].rearrange("(c p) t -> p c t", p=P),
                )
                for cx in range(NTC):
                    nc.sync.dma_start(
                        vaug[:, cx * 4 : (cx + 1) * 4, 0:HD],
                        cc_kv_g[cx * P : (cx + 1) * P, TC : 2 * TC].rearrange(
                            "p (q d) -> p q d", d=HD
                        ),
                    )

            def emit_outproj3():
                c = NTC - 1
                gT = slabp.tile([P, ND, TC], BF16, tag="slab", name="gT3")
                ots = [ot for ot in range(ND) if ot % HL < 4] + [
                    ot for ot in range(ND) if ot % HL >= 4
                ]
                for ot in ots:
                    r, lh = divmod(ot, HL)
                    half, lh2 = divmod(lh, 4)
                    nc.sync.dma_start(
                        gT[:, ot, :],
                        cc_g3[half][r * HH + lh2 * P : r * HH + (lh2 + 1) * P, :],
                    )
                for tt in range(TC // P):
                    for dc in range(OL // TC):
                        ps = psump.tile(
                            [P, TC], F32, tag="tr", bufs=2, name=f"pso3_{tt}_{dc}"
                        )
                        for i, ot in enumerate(ots):
                            nc.tensor.matmul(
                                ps[:],
                                gT[:, ot, tt * P : (tt + 1) * P],
                                woT[:, ot, dc * TC : (dc + 1) * TC],
                                start=(i == 0),
                                stop=(i == ND - 1),
                            )
                        osb = tmpp.tile([P, TC], F32, tag="osb", bufs=1, name=f"osb3{tt}{dc}")
                        nc.vector.tensor_tensor(
                            osb[:],
                            ps[:],
                            bo_bc[:, dc * TC : (dc + 1) * TC],
                            mybir.AluOpType.add,
                        )
                        nc.sync.dma_start(
                            out_ext[
                                c * TC + tt * P : c * TC + (tt + 1) * P,
                                dc * TC : (dc + 1) * TC,
                            ],
                            osb[:],
                        )

            def emit_outproj(c):
                gT = slabp.tile([P, ND, TC], BF16, tag="slab", name=f"gT{c}")
                for ot in range(ND):
                    r, lh = divmod(ot, HL)
                    nc.sync.dma_start(
                        gT[:, ot, :],
                        cc_g[c][r * OL + lh * P : r * OL + (lh + 1) * P, :],
                    )
                for tt in range(TC // P):
                    for dc in range(OL // TC):
                        ps = psump.tile(
                            [P, TC], F32, tag="tr", bufs=2, name=f"pso{c}_{tt}_{dc}"
                        )
                        for ot in range(ND):
                            nc.tensor.matmul(
                                ps[:],
                                gT[:, ot, tt * P : (tt + 1) * P],
                                woT[:, ot, dc * TC : (dc + 1) * TC],
                                start=(ot == 0),
                                stop=(ot == ND - 1),
                            )
                        osb = tmpp.tile([P, TC], F32, tag="osb", bufs=1, name=f"osb{c}{tt}{dc}")
                        nc.vector.tensor_tensor(
                            osb[:],
                            ps[:],
                            bo_bc[:, dc * TC : (dc + 1) * TC],
                            mybir.AluOpType.add,
                        )
                        nc.sync.dma_start(
                            out_ext[
                                c * TC + tt * P : c * TC + (tt + 1) * P,
                                dc * TC : (dc + 1) * TC,
                            ],
                            osb[:],
                        )

            def emit_qproj(c, xT, stream_wq=False):
                qTc = qtcp.tile([P, HL, TC], BF16, tag="qTc", name=f"qTc{c}")
                for ot in range(HL):
                    if stream_wq:
                        if ot + 2 < HL:
                            nat_q[ot + 2] = natp.tile(
                                [P, D], BF16, tag="nat", name=f"natq{ot + 2}"
                            )
                            nc.gpsimd.dma_start(
                                nat_q[ot + 2][:],
                                wq_ext[(ot + 2) * P : (ot + 3) * P, :],
                            )
                        nat = nat_q[ot]
                        for g in range(0, ND, 4):
                            pst = psump.tile(
                                [P, 4 * P], BF16, tag="attn", bufs=4,
                                name=f"ptq{ot}{g}",
                            )
                            for j in range(4):
                                nc.tensor.transpose(
                                    pst[:, j * P : (j + 1) * P],
                                    nat[:, (g + j) * P : (g + j + 1) * P],
                                    ident[:],
                                )
                            nc.vector.tensor_copy(
                                wqT[:, g : g + 4, ot * P : (ot + 1) * P],
                                pst[:].rearrange("p (g t) -> p g t", g=4),
                            )
                    ps = psump.tile(
                        [P, TC], F32, tag="mm512", bufs=2, name=f"psq{c}_{ot}"
                    )
                    for dt in range(ND):
                        nc.tensor.matmul(
                            ps[:],
                            wqT[:, dt, ot * P : (ot + 1) * P],
                            xT[:, dt, :],
                            start=(dt == 0),
                            stop=(dt == ND - 1),
                        )
                    nc.vector.tensor_scalar_add(
                        qTc[:, ot, :], ps[:], bq_sb[:, ot : ot + 1]
                    )
                return qTc

            def emit_attention(c, qTc, attnT_c):
                nkt = (c + 1) * (TC // P)  # causal: k-tiles 0..nkt-1
                for h in range(HL):
                    att_ps = [
                        psump.tile(
                            [P, HD + 1], F32, tag="attn", bufs=4, name=f"att{c}_{h}_{qt}"
                        )
                        for qt in range(TC // P)
                    ]
                    for kt in range(nkt):
                        # Band tiles only need columns tq >= (kt-4c)*128; after
                        # trimming, the causal pattern is always the diagonal.
                        off = (kt - (nkt - 4)) * P if kt >= nkt - 4 else 0
                        ne = TC - off
                        ps_s = psump.tile(
                            [P, TC], F32, tag="mm512", bufs=2, name=f"pss{c}_{h}_{kt}"
                        )
                        nc.tensor.matmul(
                            ps_s[:, :ne],
                            kT[:, kt * P : (kt + 1) * P],
                            qTc[:, h, off:TC],
                            start=True,
                            stop=True,
                        )
                        es = esbp.tile([P, TC], BF16, tag="esb", name=f"es{c}_{h}_{kt}")
                        nc.scalar.activation(
                            es[:, :ne],
                            ps_s[:, :ne],
                            mybir.ActivationFunctionType.Exp,
                            scale=SCALE,
                        )
                        if kt >= nkt - 4:
                            # Zero weights where k > q (pure diagonal after trim).
                            nc.vector.tensor_tensor(
                                es[:, :ne], es[:, :ne], mask0[:, :ne],
                                mybir.AluOpType.mult,
                            )
                        for qt in range(TC // P):
                            tqi = c * (TC // P) + qt
                            if kt > tqi:
                                continue
                            nc.tensor.matmul(
                                att_ps[qt][:],
                                es[:, qt * P - off : (qt + 1) * P - off],
                                vaug[:, kt, :],
                                start=(kt == 0),
                                stop=(kt == tqi),
                            )
                    psta = psump.tile(
                        [P, 4 * P], BF16, tag="attn", bufs=4, name=f"psta{c}{h}"
                    )
                    for qt in range(TC // P):
                        recip = tmpp.tile([P, 1], F32, tag="recip", bufs=2, name=f"rc{c}{h}{qt}")
                        nc.vector.reciprocal(recip[:], att_ps[qt][:, HD : HD + 1])
                        attn_sb = tmpp.tile(
                            [P, P], BF16, tag="attn_sb", bufs=2, name=f"asb{c}{h}{qt}"
                        )
                        nc.vector.tensor_scalar_mul(
                            attn_sb[:], att_ps[qt][:, 0:HD], recip[:]
                        )
                        nc.tensor.transpose(
                            psta[:, qt * P : (qt + 1) * P], attn_sb[:], ident[:]
                        )
                    nc.vector.tensor_copy(attnT_c[:, h, :], psta[:])
                    if c == NTC - 1 and h == 3:
                        nc.sync.dma_start(
                            cc_in3[0][:, :].rearrange("(h p) t -> p h t", p=P),
                            attnT_c[:, 0:4, :],
                        )
                        nc.gpsimd.collective_compute(
                            "AllGather",
                            mybir.AluOpType.bypass,
                            replica_groups=[[0, 1, 2, 3], [4, 5, 6, 7]],
                            ins=[cc_in3[0][:, :].opt()],
                            outs=[cc_g3[0][:, :].opt()],
                        )
                        emit_outproj(0)
                    if c == NTC - 1 and h == 5:
                        emit_outproj(1)
                    if c == NTC - 1 and h == 7:
                        emit_outproj(2)

            def ship_attn(c, attnT_c):
                if c < NTC - 1:
                    nc.sync.dma_start(
                        cc_in[c][:, :].rearrange("(h p) t -> p h t", p=P), attnT_c[:]
                    )
                    nc.gpsimd.collective_compute(
                        "AllGather",
                        mybir.AluOpType.bypass,
                        replica_groups=[[0, 1, 2, 3], [4, 5, 6, 7]],
                        ins=[cc_in[c][:, :].opt()],
                        outs=[cc_g[c][:, :].opt()],
                    )
                else:
                    nc.sync.dma_start(
                        cc_in3[1][:, :].rearrange("(h p) t -> p h t", p=P),
                        attnT_c[:, 4:HL, :],
                    )
                    nc.gpsimd.collective_compute(
                        "AllGather",
                        mybir.AluOpType.bypass,
                        replica_groups=[[0, 1, 2, 3], [4, 5, 6, 7]],
                        ins=[cc_in3[1][:, :].opt()],
                        outs=[cc_g3[1][:, :].opt()],
                    )
                    emit_outproj3()

            # ================= chunk 0 =================
            # x chunk 0 on-chip: scalar f32 loads + DVE cast + PE transpose.
            xT0 = slabp.tile([P, ND, TC], BF16, tag="slab", name="xT0")
            for tb in range(TC // P):
                pieces = load_cast_x(tb * P, f"x0{tb}")
                transpose_pieces(pieces, xT0, tb * P, f"x0{tb}")
            # x chunk 1 bounce pipeline: scalar loads + casts + scalar stores.
            for r in range(TC // P, 2 * (TC // P)):
                load_cast_x(r * P, f"xs{r}", store_dst=x_bf)
            # xT1 DMA-transpose: first big item on the sync ring.
            xT1 = slabp.tile([P, ND, TC], BF16, tag="slab", name="xT1")
            nc.sync.dma_start(xT1[:], x_bf[TC : 2 * TC, :], transpose=True)
            # kv unpack after xT1 in the sync FIFO (it waits on the gather).
            emit_unpack_kv()

            qTc0 = emit_qproj(0, xT0, stream_wq=True)

            # x chunks 2-3 bounce pipelines (scalar ring).
            for r in range(2 * (TC // P), T // P):
                load_cast_x(r * P, f"xs{r}", store_dst=x_bf)
            # wo pipeline: gpsimd cast loads (after wq on that ring), sync
            # stores — interleaved so each store precedes its buffer's reuse.
            for r in range(OL // P):
                nat = natp.tile([P, D], BF16, tag="nat", name=f"natwo{r}")
                nc.gpsimd.dma_start(nat[:], wo_ext[r * P : (r + 1) * P, :])
                nc.sync.dma_start(wo_bf[r * P : (r + 1) * P, :], nat[:])

            attnT_0 = atcp.tile([P, HL, TC], BF16, tag="atc", name="attnT0")
            emit_attention(0, qTc0, attnT_0)
            ship_attn(0, attnT_0)

            # ================= chunk 1 =================
            # Prefetch xT2 and xT3 (sync ring; x_bf rows land early enough).
            xT2 = slabp.tile([P, ND, TC], BF16, tag="slab", name="xT2")
            nc.sync.dma_start(xT2[:], x_bf[2 * TC : 3 * TC, :], transpose=True)
            xT3 = slabp.tile([P, ND, TC], BF16, tag="slab", name="xT3")
            nc.sync.dma_start(xT3[:], x_bf[3 * TC : 4 * TC, :], transpose=True)

            qTc1 = emit_qproj(1, xT1)
            attnT_1 = atcp.tile([P, HL, TC], BF16, tag="atc", name="attnT1")
            emit_attention(1, qTc1, attnT_1)
            ship_attn(1, attnT_1)

            # ================= chunk 2 (+ hoisted Q3) =================
            qTc2 = emit_qproj(2, xT2)
            qTc3 = emit_qproj(3, xT3)
            # wqT is dead now: start the woT DMA-transposes (sync ring).
            nc.sync.dma_start(
                woT[:, :, 0 : OL // 2], wo_bf[0 : OL // 2, :], transpose=True
            )
            nc.sync.dma_start(
                woT[:, :, OL // 2 : OL], wo_bf[OL // 2 : OL, :], transpose=True
            )
            attnT_2 = atcp.tile([P, HL, TC], BF16, tag="atc", name="attnT2")
            emit_attention(2, qTc2, attnT_2)
            ship_attn(2, attnT_2)

            # ================= chunk 3 =================
            attnT_3 = atcp.tile([P, HL, TC], BF16, tag="atc", name="attnT3")
            emit_attention(3, qTc3, attnT_3)
            ship_attn(3, attnT_3)

    nc.compile()
    return nc


def kernel(x, wq_w, wq_b, wk_w, wk_b, wv_w, wv_b, wo_w, wo_b):
    global LAST_RESULT
    if "nc" not in _CACHE:
        _CACHE["nc"] = build_nc()
    nc = _CACHE["nc"]

    f32 = np.float32
    x = np.asarray(x, f32)
    in_maps = []
    for c in range(NCORES):
        b, g = divmod(c, TPG)
        sl = slice(OL * g, OL * (g + 1))
        in_maps.append(
            {
                "x": np.ascontiguousarray(x[b]),
                "xkv": np.ascontiguousarray(x[b][TC * g : TC * (g + 1)]),
                "wq": np.ascontiguousarray(np.asarray(wq_w, f32)[sl]),
                "bq": np.ascontiguousarray(np.asarray(wq_b, f32)[sl]),
                "wk": np.ascontiguousarray(np.asarray(wk_w, f32)),
                "bk": np.ascontiguousarray(np.asarray(wk_b, f32)),
                "wv": np.ascontiguousarray(np.asarray(wv_w, f32)),
                "bv": np.ascontiguousarray(np.asarray(wv_b, f32)),
                "wo": np.ascontiguousarray(np.asarray(wo_w, f32)[sl]),
                "bo": np.ascontiguousarray(np.asarray(wo_b, f32)[sl]),
            }
        )

    res = run_bass_kernel_spmd(nc, in_maps, core_ids=list(range(NCORES)))
    LAST_RESULT = res

    out = np.empty((B, T, D), dtype=f32)
    for c in range(NCORES):
        b, g = divmod(c, TPG)
        out[b, :, OL * g : OL * (g + 1)] = res.results[c]["out"]
    return out


# revision 15
# speedup vs baseline: 1.2205x; 1.2205x over previous
"""Distributed MQA causal attention for TRN2 (8 NeuronCores).

Sharding: 8 cores = 2 (batch) x 4 (head-group tensor parallel).
Core c handles batch b=c//4, head group g=c%4 (8 heads, o-slice of 1024).

K/V projection is t-sharded across the 4 cores of a batch group: each core
receives an extra input `xkv` (its 512-row slice of x), computes K/V for
that chunk once, and one AllGather assembles the full K^T / V for all
chunks (vs. every core computing all of K/V).

After attention, the per-core attn^T chunks are AllGather-ed (groups of 4)
and each core computes a 1024-wide column slice of the output projection.
The chunk-3 Q projection is hoisted into the chunk-2 body so wq^T dies
early; wo^T (aliased on wq^T) then loads during chunk-2 attention and the
output projections overlap chunk-3 attention instead of serializing after.

All matmuls run in bf16 (f32 accumulation in PSUM).  DMA ring plan (each
DGE ring is FIFO, so each gets a stream whose deadlines are monotone):
  gpsimd (SWDGE, casts): xkv, wk, wv, wq (streamed), wo
  scalar (HWDGE):        all x f32 loads + DVE bf16 casts; x_bf stores
  sync   (HWDGE):        xT DMA-transposes, kv unpack, wo_bf stores, woT,
                         attn ships, gT loads, out writes
"""

import numpy as np

import concourse.bass as bass
import concourse.mybir as mybir
import concourse.tile as tile
from concourse import bacc
from concourse.bass_utils import run_bass_kernel_spmd
from concourse.masks import make_identity

# Problem shape (hardcoded; kernel.py must be self-contained).
B, T, D = 2, 2048, 4096
H, HD = 32, 128
NCORES, TPG = 8, 4
HL = H // TPG            # 8 local heads per core
OL = HL * HD             # 1024 local q/o dims per core
P = 128
TC = 512                 # t-chunk width (moving-dim of the big GEMMs)
NTC = T // TC            # 4
ND = D // P              # 32 contraction tiles for D
NT = T // P              # 16 k-tiles
QW = 512                 # staging width for f32 x loads
NQ = D // QW             # 8 pieces per row-tile
SCALE = float(1.0 / np.sqrt(HD))

BF16 = mybir.dt.bfloat16
F32 = mybir.dt.float32

_CACHE = {}
LAST_RESULT = None  # BassKernelResults of the most recent run (for test harness)


def build_nc():
    nc = bacc.Bacc(None, target_bir_lowering=False, num_devices=NCORES)

    x_ext = nc.declare_dram_parameter("x", [T, D], F32, isOutput=False)
    xkv_ext = nc.declare_dram_parameter("xkv", [TC, D], F32, isOutput=False)
    wq_ext = nc.declare_dram_parameter("wq", [OL, D], F32, isOutput=False)
    bq_ext = nc.declare_dram_parameter("bq", [OL], F32, isOutput=False)
    wk_ext = nc.declare_dram_parameter("wk", [HD, D], F32, isOutput=False)
    bk_ext = nc.declare_dram_parameter("bk", [HD], F32, isOutput=False)
    wv_ext = nc.declare_dram_parameter("wv", [HD, D], F32, isOutput=False)
    bv_ext = nc.declare_dram_parameter("bv", [HD], F32, isOutput=False)
    wo_ext = nc.declare_dram_parameter("wo", [OL, D], F32, isOutput=False)
    bo_ext = nc.declare_dram_parameter("bo", [OL], F32, isOutput=False)
    out_ext = nc.declare_dram_parameter("out", [T, OL], F32, isOutput=True)

    with tile.TileContext(nc) as tc:
        with (
            tc.tile_pool(name="consts", bufs=1) as consts,
            tc.tile_pool(name="wpool", bufs=1) as wpool,
            tc.tile_pool(name="wsmall", bufs=1) as wsmall,
            tc.tile_pool(name="slab", bufs=2) as slabp,
            tc.tile_pool(name="nat", bufs=2) as natp,
            tc.tile_pool(name="big", bufs=1) as bigp,
            tc.tile_pool(name="qtc", bufs=2) as qtcp,
            tc.tile_pool(name="atc", bufs=1) as atcp,
            tc.tile_pool(name="esb", bufs=2) as esbp,
            tc.tile_pool(name="tmp", bufs=3) as tmpp,
            tc.tile_pool(name="psum", bufs=1, space="PSUM") as psump,
            tc.tile_pool(name="dram", bufs=1, space="DRAM") as dram,
        ):
            # ---- Constants (tiny, first so gpsimd builds them before casts)
            ident = consts.tile([P, P], BF16)
            make_identity(nc, ident[:])
            # Diagonal causal 0/1 mask: mask0[x, y] = 1 if y >= x else 0.
            mask0 = consts.tile([P, TC], BF16, name="mask0")
            nc.gpsimd.memset(mask0[:], 1.0)
            nc.gpsimd.affine_select(
                out=mask0[:],
                in_=mask0[:],
                pattern=[[1, TC]],
                compare_op=mybir.AluOpType.is_ge,
                fill=0.0,
                base=0,
                channel_multiplier=-1,
            )
            bq_sb = consts.tile([P, HL], F32)
            nc.sync.dma_start(bq_sb[:], bq_ext[:].rearrange("(o p) -> p o", p=P))
            bk_sb = consts.tile([P, 1], F32)
            nc.sync.dma_start(bk_sb[:], bk_ext[:].rearrange("(o p) -> p o", p=P))
            bv_sb = consts.tile([P, 1], F32)
            nc.sync.dma_start(bv_sb[:], bv_ext[:].rearrange("(o p) -> p o", p=P))
            bo_row = natp.tile([1, OL], BF16, tag="nat", name="bo_row")
            nc.gpsimd.dma_start(bo_row[:], bo_ext[None, :])
            bo_bc = consts.tile([P, OL], BF16)
            nc.gpsimd.partition_broadcast(bo_bc[:], bo_row[:])

            # Persistent attention operands.
            kT = bigp.tile([P, T], BF16, name="kT")               # [hd, t]
            vaug = bigp.tile([P, NT, HD + 1], BF16, name="vaug")  # [tk, kt, 129]
            nc.vector.memset(vaug[:, :, HD : HD + 1], 1.0)

            wqT = wpool.tile([P, ND, OL], BF16, tag="bigw", name="wqT")
            wkT = wsmall.tile([P, ND, HD], BF16, name="wkT")
            wvT = wsmall.tile([P, ND, HD], BF16, name="wvT")

            # ---- gpsimd cast loads (SWDGE) + PE transpose --------------------
            def load_T_gpsimd(src_ext, rows, dstT, col0, what):
                for blk in range(rows // P):
                    nat = natp.tile([P, D], BF16, tag="nat", name=f"nat_{what}{blk}")
                    nc.gpsimd.dma_start(nat[:], src_ext[blk * P : (blk + 1) * P, :])
                    for g in range(0, ND, 4):
                        pst = psump.tile(
                            [P, 4 * P], BF16, tag="attn", bufs=4, name=f"ptr_{what}{blk}{g}"
                        )
                        for j in range(4):
                            nc.tensor.transpose(
                                pst[:, j * P : (j + 1) * P],
                                nat[:, (g + j) * P : (g + j + 1) * P],
                                ident[:],
                            )
                        nc.vector.tensor_copy(
                            dstT[:, g : g + 4, col0 + blk * P : col0 + (blk + 1) * P],
                            pst[:].rearrange("p (g t) -> p g t", g=4),
                        )

            # ---- gpsimd cast load + sync store (x bounce rows) ---------------
            def cast_to_scratch(src_ext, dst, r0, r1, what):
                for i, r in enumerate(range(r0, r1, P)):
                    t_ = natp.tile([P, D], BF16, tag="nat", name=f"cs_{what}{i}")
                    nc.gpsimd.dma_start(t_[:], src_ext[r : r + P, :])
                    nc.sync.dma_start(dst[r : r + P, :], t_[:])

            # ---- DRAM bf16 scratch for late operands (x1-3, wo) --------------
            x_bf = dram.tile([T, D], BF16)
            wo_bf = dram.tile([OL, D], BF16)

            # KV AllGather buffers: in [128, 1024] = [kT piece | v nat piece]
            cc_kv_in = dram.tile([P, 2 * TC], BF16, name="cc_kv_in")
            cc_kv_g = dram.tile([TPG * P, 2 * TC], BF16, name="cc_kv_g")

            # attn AllGather buffers, one per t-chunk (column-sliced attn^T).
            cc_in = [dram.tile([OL, TC], BF16, name=f"cc_in{c}") for c in range(NTC)]
            cc_g = [
                dram.tile([TPG * OL, TC], BF16, name=f"cc_g{c}") for c in range(NTC)
            ]
            HH = OL // 2  # 512 rows = 4 heads
            cc_in3 = [dram.tile([HH, TC], BF16, name=f"cc_in3{i}") for i in range(2)]
            cc_g3 = [dram.tile([TPG * HH, TC], BF16, name=f"cc_g3{i}") for i in range(2)]

            woT = wpool.tile([P, ND, OL], BF16, tag="bigw", name="woT")

            # ========== Stage A: sharded K/V projection + gather ==============
            xkvT = slabp.tile([P, ND, TC], BF16, tag="slab", name="xkvT")
            load_T_gpsimd(xkv_ext, TC, xkvT, 0, "xkv")
            load_T_gpsimd(wk_ext, HD, wkT, 0, "wk")
            load_T_gpsimd(wv_ext, HD, wvT, 0, "wv")

            for which, wT, b_sb in (("k", wkT, bk_sb), ("v", wvT, bv_sb)):
                ps = psump.tile([P, TC], F32, tag="mm512", bufs=2, name=f"pskv_{which}")
                for dt in range(ND):
                    nc.tensor.matmul(
                        ps[:],
                        wT[:, dt, :],
                        xkvT[:, dt, :],
                        start=(dt == 0),
                        stop=(dt == ND - 1),
                    )
                if which == "k":
                    kpc = tmpp.tile([P, TC], BF16, tag="vt", bufs=1, name="kpc")
                    nc.vector.tensor_scalar_add(kpc[:], ps[:], b_sb[:])
                    nc.sync.dma_start(cc_kv_in[:, 0:TC], kpc[:])
                else:
                    vt = tmpp.tile([P, TC], BF16, tag="vt", bufs=1, name="vt_kv")
                    nc.vector.tensor_scalar_add(vt[:], ps[:], b_sb[:])
                    pstv = psump.tile([P, 4 * P], BF16, tag="attn", bufs=4, name="pstv")
                    for jj in range(TC // P):
                        nc.tensor.transpose(
                            pstv[:, jj * P : (jj + 1) * P],
                            vt[:, jj * P : (jj + 1) * P],
                            ident[:],
                        )
                    vnat = tmpp.tile([P, TC], BF16, tag="vt", bufs=1, name="vnat")
                    nc.vector.tensor_copy(vnat[:], pstv[:])
                    nc.sync.dma_start(cc_kv_in[:, TC : 2 * TC], vnat[:])

            nc.gpsimd.collective_compute(
                "AllGather",
                mybir.AluOpType.bypass,
                replica_groups=[[0, 1, 2, 3], [4, 5, 6, 7]],
                ins=[cc_kv_in[:, :].opt()],
                outs=[cc_kv_g[:, :].opt()],
            )

            def emit_unpack_kv():
                # kT[hd, t] and vaug[tk, kt, hd] from the gathered buffer.
                nc.sync.dma_start(
                    kT[:, :].rearrange("p (c t) -> p c t", c=NTC),
                    cc_kv_g[:, 0:TC].rearrange("(c p) t -> p c t", p=P),
                )
                for cx in range(NTC):
                    nc.sync.dma_start(
                        vaug[:, cx * 4 : (cx + 1) * 4, 0:HD],
                        cc_kv_g[cx * P : (cx + 1) * P, TC : 2 * TC].rearrange(
                            "p (q d) -> p q d", d=HD
                        ),
                    )

            def emit_outproj3():
                c = NTC - 1
                gT = slabp.tile([P, ND, TC], BF16, tag="slab", name="gT3")
                ots = [ot for ot in range(ND) if ot % HL < 4] + [
                    ot for ot in range(ND) if ot % HL >= 4
                ]
                for ot in ots:
                    r, lh = divmod(ot, HL)
                    half, lh2 = divmod(lh, 4)
                    nc.sync.dma_start(
                        gT[:, ot, :],
                        cc_g3[half][r * HH + lh2 * P : r * HH + (lh2 + 1) * P, :],
                    )
                for tt in range(TC // P):
                    for dc in range(OL // TC):
                        ps = psump.tile(
                            [P, TC], F32, tag="tr", bufs=2, name=f"pso3_{tt}_{dc}"
                        )
                        for i, ot in enumerate(ots):
                            nc.tensor.matmul(
                                ps[:],
                                gT[:, ot, tt * P : (tt + 1) * P],
                                woT[:, ot, dc * TC : (dc + 1) * TC],
                                start=(i == 0),
                                stop=(i == ND - 1),
                            )
                        osb = tmpp.tile([P, TC], F32, tag="osb", bufs=1, name=f"osb3{tt}{dc}")
                        nc.vector.tensor_tensor(
                            osb[:],
                            ps[:],
                            bo_bc[:, dc * TC : (dc + 1) * TC],
                            mybir.AluOpType.add,
                        )
                        nc.sync.dma_start(
                            out_ext[
                                c * TC + tt * P : c * TC + (tt + 1) * P,
                                dc * TC : (dc + 1) * TC,
                            ],
                            osb[:],
                        )

            def emit_outproj(c):
                gT = slabp.tile([P, ND, TC], BF16, tag="slab", name=f"gT{c}")
                for ot in range(ND):
                    r, lh = divmod(ot, HL)
                    nc.sync.dma_start(
                        gT[:, ot, :],
                        cc_g[c][r * OL + lh * P : r * OL + (lh + 1) * P, :],
                    )
                for tt in range(TC // P):
                    for dc in range(OL // TC):
                        ps = psump.tile(
                            [P, TC], F32, tag="tr", bufs=2, name=f"pso{c}_{tt}_{dc}"
                        )
                        for ot in range(ND):
                            nc.tensor.matmul(
                                ps[:],
                                gT[:, ot, tt * P : (tt + 1) * P],
                                woT[:, ot, dc * TC : (dc + 1) * TC],
                                start=(ot == 0),
                                stop=(ot == ND - 1),
                            )
                        osb = tmpp.tile([P, TC], F32, tag="osb", bufs=1, name=f"osb{c}{tt}{dc}")
                        nc.vector.tensor_tensor(
                            osb[:],
                            ps[:],
                            bo_bc[:, dc * TC : (dc + 1) * TC],
                            mybir.AluOpType.add,
                        )
                        nc.sync.dma_start(
                            out_ext[
                                c * TC + tt * P : c * TC + (tt + 1) * P,
                                dc * TC : (dc + 1) * TC,
                            ],
                            osb[:],
                        )

            def emit_qproj(c, xT, stream_wq=False):
                qTc = qtcp.tile([P, HL, TC], BF16, tag="qTc", name=f"qTc{c}")
                for ot in range(HL):
                    if stream_wq:
                        if ot + 2 < HL:
                            nat_q[ot + 2] = natp.tile(
                                [P, D], BF16, tag="nat", name=f"natq{ot + 2}"
                            )
                            nc.gpsimd.dma_start(
                                nat_q[ot + 2][:],
                                wq_ext[(ot + 2) * P : (ot + 3) * P, :],
                            )
                        nat = nat_q[ot]
                        for g in range(0, ND, 4):
                            pst = psump.tile(
                                [P, 4 * P], BF16, tag="attn", bufs=4,
                                name=f"ptq{ot}{g}",
                            )
                            for j in range(4):
                                nc.tensor.transpose(
                                    pst[:, j * P : (j + 1) * P],
                                    nat[:, (g + j) * P : (g + j + 1) * P],
                                    ident[:],
                                )
                            nc.vector.tensor_copy(
                                wqT[:, g : g + 4, ot * P : (ot + 1) * P],
                                pst[:].rearrange("p (g t) -> p g t", g=4),
                            )
                    ps = psump.tile(
                        [P, TC], F32, tag="mm512", bufs=2, name=f"psq{c}_{ot}"
                    )
                    for dt in range(ND):
                        nc.tensor.matmul(
                            ps[:],
                            wqT[:, dt, ot * P : (ot + 1) * P],
                            xT[:, dt, :],
                            start=(dt == 0),
                            stop=(dt == ND - 1),
                        )
                    nc.vector.tensor_scalar_add(
                        qTc[:, ot, :], ps[:], bq_sb[:, ot : ot + 1]
                    )
                return qTc

            def emit_attention(c, qTc, attnT_c):
                nkt = (c + 1) * (TC // P)  # causal: k-tiles 0..nkt-1
                for h in range(HL):
                    att_ps = [
                        psump.tile(
                            [P, HD + 1], F32, tag="attn", bufs=4, name=f"att{c}_{h}_{qt}"
                        )
                        for qt in range(TC // P)
                    ]
                    for kt in range(nkt):
                        # Band tiles only need columns tq >= (kt-4c)*128; after
                        # trimming, the causal pattern is always the diagonal.
                        off = (kt - (nkt - 4)) * P if kt >= nkt - 4 else 0
                        ne = TC - off
                        ps_s = psump.tile(
                            [P, TC], F32, tag="mm512", bufs=2, name=f"pss{c}_{h}_{kt}"
                        )
                        nc.tensor.matmul(
                            ps_s[:, :ne],
                            kT[:, kt * P : (kt + 1) * P],
                            qTc[:, h, off:TC],
                            start=True,
                            stop=True,
                        )
                        es = esbp.tile([P, TC], BF16, tag="esb", name=f"es{c}_{h}_{kt}")
                        nc.scalar.activation(
                            es[:, :ne],
                            ps_s[:, :ne],
                            mybir.ActivationFunctionType.Exp,
                            scale=SCALE,
                        )
                        if kt >= nkt - 4:
                            # Zero weights where k > q (pure diagonal after trim).
                            nc.vector.tensor_tensor(
                                es[:, :ne], es[:, :ne], mask0[:, :ne],
                                mybir.AluOpType.mult,
                            )
                        for qt in range(TC // P):
                            tqi = c * (TC // P) + qt
                            if kt > tqi:
                                continue
                            nc.tensor.matmul(
                                att_ps[qt][:],
                                es[:, qt * P - off : (qt + 1) * P - off],
                                vaug[:, kt, :],
                                start=(kt == 0),
                                stop=(kt == tqi),
                            )
                    psta = psump.tile(
                        [P, 4 * P], BF16, tag="attn", bufs=4, name=f"psta{c}{h}"
                    )
                    for qt in range(TC // P):
                        recip = tmpp.tile([P, 1], F32, tag="recip", bufs=2, name=f"rc{c}{h}{qt}")
                        nc.vector.reciprocal(recip[:], att_ps[qt][:, HD : HD + 1])
                        attn_sb = tmpp.tile(
                            [P, P], BF16, tag="attn_sb", bufs=2, name=f"asb{c}{h}{qt}"
                        )
                        nc.vector.tensor_scalar_mul(
                            attn_sb[:], att_ps[qt][:, 0:HD], recip[:]
                        )
                        nc.tensor.transpose(
                            psta[:, qt * P : (qt + 1) * P], attn_sb[:], ident[:]
                        )
                    nc.vector.tensor_copy(attnT_c[:, h, :], psta[:])
                    if c == NTC - 1 and h == 3:
                        nc.sync.dma_start(
                            cc_in3[0][:, :].rearrange("(h p) t -> p h t", p=P),
                            attnT_c[:, 0:4, :],
                        )
                        nc.gpsimd.collective_compute(
                            "AllGather",
                            mybir.AluOpType.bypass,
                            replica_groups=[[0, 1, 2, 3], [4, 5, 6, 7]],
                            ins=[cc_in3[0][:, :].opt()],
                            outs=[cc_g3[0][:, :].opt()],
                        )
                        emit_outproj(0)
                    if c == NTC - 1 and h == 5:
                        emit_outproj(1)
                    if c == NTC - 1 and h == 7:
                        emit_outproj(2)

            def ship_attn(c, attnT_c):
                if c < NTC - 1:
                    nc.sync.dma_start(
                        cc_in[c][:, :].rearrange("(h p) t -> p h t", p=P), attnT_c[:]
                    )
                    nc.gpsimd.collective_compute(
                        "AllGather",
                        mybir.AluOpType.bypass,
                        replica_groups=[[0, 1, 2, 3], [4, 5, 6, 7]],
                        ins=[cc_in[c][:, :].opt()],
                        outs=[cc_g[c][:, :].opt()],
                    )
                else:
                    nc.sync.dma_start(
                        cc_in3[1][:, :].rearrange("(h p) t -> p h t", p=P),
                        attnT_c[:, 4:HL, :],
                    )
                    nc.gpsimd.collective_compute(
                        "AllGather",
                        mybir.AluOpType.bypass,
                        replica_groups=[[0, 1, 2, 3], [4, 5, 6, 7]],
                        ins=[cc_in3[1][:, :].opt()],
                        outs=[cc_g3[1][:, :].opt()],
                    )
                    emit_outproj3()

            # ================= chunk 0 =================
            # x chunk 0: gpsimd cast loads + PE transposes (after wk/wv on
            # the gpsimd ring, before wq).
            xT0 = slabp.tile([P, ND, TC], BF16, tag="slab", name="xT0")
            load_T_gpsimd(x_ext, TC, xT0, 0, "x0")
            # kv unpack (sync ring; waits on the gather).
            emit_unpack_kv()

            nat_q = {}
            for oo in range(2):
                nat_q[oo] = natp.tile([P, D], BF16, tag="nat", name=f"natq{oo}")
                nc.gpsimd.dma_start(nat_q[oo][:], wq_ext[oo * P : (oo + 1) * P, :])
            qTc0 = emit_qproj(0, xT0, stream_wq=True)

            # x chunks 1-3 casts (gpsimd ring after wq), then wo casts.
            cast_to_scratch(x_ext, x_bf, TC, T, "x")
            for r in range(OL // P):
                nat = natp.tile([P, D], BF16, tag="nat", name=f"natwo{r}")
                nc.gpsimd.dma_start(nat[:], wo_ext[r * P : (r + 1) * P, :])
                nc.sync.dma_start(wo_bf[r * P : (r + 1) * P, :], nat[:])
            # xT1 DMA-transpose on the sync ring (after the ch-1 stores).
            xT1 = slabp.tile([P, ND, TC], BF16, tag="slab", name="xT1")
            nc.sync.dma_start(xT1[:], x_bf[TC : 2 * TC, :], transpose=True)

            attnT_0 = atcp.tile([P, HL, TC], BF16, tag="atc", name="attnT0")
            emit_attention(0, qTc0, attnT_0)
            ship_attn(0, attnT_0)

            # ================= chunk 1 =================
            # Prefetch xT2 and xT3 (sync ring; x_bf rows land early enough).
            xT2 = slabp.tile([P, ND, TC], BF16, tag="slab", name="xT2")
            nc.sync.dma_start(xT2[:], x_bf[2 * TC : 3 * TC, :], transpose=True)
            xT3 = slabp.tile([P, ND, TC], BF16, tag="slab", name="xT3")
            nc.sync.dma_start(xT3[:], x_bf[3 * TC : 4 * TC, :], transpose=True)

            qTc1 = emit_qproj(1, xT1)
            attnT_1 = atcp.tile([P, HL, TC], BF16, tag="atc", name="attnT1")
            emit_attention(1, qTc1, attnT_1)
            ship_attn(1, attnT_1)

            # ================= chunk 2 (+ hoisted Q3) =================
            qTc2 = emit_qproj(2, xT2)
            qTc3 = emit_qproj(3, xT3)
            # wqT is dead now: start the woT DMA-transposes (sync ring).
            nc.sync.dma_start(
                woT[:, :, 0 : OL // 2], wo_bf[0 : OL // 2, :], transpose=True
            )
            nc.sync.dma_start(
                woT[:, :, OL // 2 : OL], wo_bf[OL // 2 : OL, :], transpose=True
            )
            attnT_2 = atcp.tile([P, HL, TC], BF16, tag="atc", name="attnT2")
            emit_attention(2, qTc2, attnT_2)
            ship_attn(2, attnT_2)

            # ================= chunk 3 =================
            attnT_3 = atcp.tile([P, HL, TC], BF16, tag="atc", name="attnT3")
            emit_attention(3, qTc3, attnT_3)
            ship_attn(3, attnT_3)

    nc.compile()
    return nc


def kernel(x, wq_w, wq_b, wk_w, wk_b, wv_w, wv_b, wo_w, wo_b):
    global LAST_RESULT
    if "nc" not in _CACHE:
        _CACHE["nc"] = build_nc()
    nc = _CACHE["nc"]

    f32 = np.float32
    x = np.asarray(x, f32)
    in_maps = []
    for c in range(NCORES):
        b, g = divmod(c, TPG)
        sl = slice(OL * g, OL * (g + 1))
        in_maps.append(
            {
                "x": np.ascontiguousarray(x[b]),
                "xkv": np.ascontiguousarray(x[b][TC * g : TC * (g + 1)]),
                "wq": np.ascontiguousarray(np.asarray(wq_w, f32)[sl]),
                "bq": np.ascontiguousarray(np.asarray(wq_b, f32)[sl]),
                "wk": np.ascontiguousarray(np.asarray(wk_w, f32)),
                "bk": np.ascontiguousarray(np.asarray(wk_b, f32)),
                "wv": np.ascontiguousarray(np.asarray(wv_w, f32)),
                "bv": np.ascontiguousarray(np.asarray(wv_b, f32)),
                "wo": np.ascontiguousarray(np.asarray(wo_w, f32)[sl]),
                "bo": np.ascontiguousarray(np.asarray(wo_b, f32)[sl]),
            }
        )

    res = run_bass_kernel_spmd(nc, in_maps, core_ids=list(range(NCORES)))
    LAST_RESULT = res

    out = np.empty((B, T, D), dtype=f32)
    for c in range(NCORES):
        b, g = divmod(c, TPG)
        out[b, :, OL * g : OL * (g + 1)] = res.results[c]["out"]
    return out
